# revision 67
# baseline (speedup 1.0000x reference)
"""GCN forward (2x graph-conv + global max-pool + linear) on 8 TRN2 NeuronCores.

Reference computation (N=16384 nodes, 256 feats, 64 hid):
    h1 = relu(adj @ (x @ W1) + b1)          [N, 64]
    h2 = adj @ (h1 @ W2) + b2               [N, 2]
    out = max(h2, axis=0) @ W3.T + b3       [1, 1, 1]

Distribution: row-shard adj over the 8 cores (core c owns output rows
[c*2048, (c+1)*2048)).  Each core:
  stage 1: Delta = bf16(16x)@bf16(W1) - m2, stored fp8  (replicated, tiny)
  pass A : h1T' = Delta.T @ adjT_fp8 + mt.T@rsum        [64, 2048] scaled
           bias/relu fused on psum evacuation (exact descale via act scale)
  stage 3: g_local = h1 @ W2 (fp32); delta_g = g_local - c
  AllGather delta_g (split per strip-pair) -> on-device collective
  pass B : h2T' = delta_g.T @ adjT_fp8 + ct.T@rsum      4x column-packed in
           one PSUM bank via tile_position; per-i-chunk max -> [128, 1]
Host: unpack/max over strips and cores, + b2, @ W3.T + b3.

Perf structure (~224-238us vs the 305us baseline; run-to-run spread is
cross-core collective skew; see comments inline):
  - adj streams as fp8e4m3 in 512 KiB tiles; k-chunks are grouped
    kg = h*8 + cc (cc = owning core, h = half) so that strip-pair `a`
    of every core produces exactly the h=a delta_g chunks -> the
    AllGather is split per pair; AG0 completes under pass-A pair 1.
  - x and W1 also stream as fp8 (the sidecars cancel the quantization
    systematics exactly); stage 1 and pass A use DoubleRow matmuls
    (fp8 fast path, 256-deep contraction per instruction).
  - pass A consumes kg groups so its first matmul data-depends on
    stage 1's LAST output chunk: stage 1 (fp8 M=128) and pass A never
    interleave on the PE (interleaving different matmul flavors drops
    the fp8 double-pump rate ~3x).  Each pair ends on CACHED groups so
    the DMA ring pre-streams pass B's first tiles during the tail.
  - the whole h=1 adj half (32 tiles = 16 MiB) stays resident in SBUF
    between the passes (24 tiles dedicated + 8 aliasing the dead xt
    buffer), cutting the 64 MiB/core adj stream to ~52 MiB.  Pass B
    opens on STREAMED h0 groups (AG0 long done) so the DMA engines
    never idle while AG1 completes; the AG1-gated h1 half runs purely
    from cache at the end.
  - HW-DGE copies are serialized round-robin onto 8 semaphore lanes;
    every latency-critical small copy (g / gf bounces) rides the SWDGE
    (Pool-engine) path instead so it is never queued behind the adj
    stream, and consts are issued before xt before adj so each lane
    serves them in dependency order.

fp8 noise is harmless because both passes compute the large mean
component exactly in fp32 via host-side sidecars:
  - rsum: exact f32 row-sums of adj (the only O(N^2) host work),
  - mt/ct: column-means, with exact cancellation of every quantization
    systematic (host simulates the device's bf16/fp8 quantization
    exactly); only sqrt(N)-damped random noise survives.
"""

import os
import sys

sys.path.insert(0, "/opt/trn_rl_repo")

import numpy as np
import ml_dtypes


def _install_ntff_hook_shim():
    """The image's `antenv` lacks `axon_hooks`, which bass_utils imports for
    trace=True under axon. Provide it, wired to the PJRT .so's NRT-profile
    C ABI (same thing trn_boot would have registered)."""
    import types
    if "antenv.axon_hooks" in sys.modules:
        return
    try:
        import antenv  # noqa: F401
        from trn_agent_boot.trn_boot import _ntff_profile_via_ctypes
        mod = types.ModuleType("antenv.axon_hooks")
        _state = {"hook": _ntff_profile_via_ctypes("/opt/axon/libaxon_pjrt.so")}
        mod.set_axon_ntff_profile_hook = lambda h: _state.update(hook=h)
        mod.get_axon_ntff_profile_hook = lambda: _state["hook"]
        sys.modules["antenv.axon_hooks"] = mod
    except Exception:
        pass


_install_ntff_hook_shim()

import concourse.bass as bass
import concourse.mybir as mybir
import concourse.tile as tile
from concourse import bacc
from concourse.bass_utils import run_bass_kernel_spmd

BF16_NP = ml_dtypes.bfloat16
FP8_NP = ml_dtypes.float8_e4m3

P = 128          # partition dim
N_CORES = 8
N_NODES = 16384
N_FEAT = 256
N_HID = 64


class Cfg:
    def __init__(self, n=N_NODES, n_feat=N_FEAT, n_hid=N_HID, n_cores=N_CORES,
                 iw=512, kpg=8, mpg=8, adj_bufs=6, n_cache_kg=4,
                 sa=21, sd=10, sx=4):
        self.n, self.n_feat, self.n_hid, self.n_cores = n, n_feat, n_hid, n_cores
        self.rows = n // n_cores       # output rows per core
        self.iw = iw                   # i-tile width (psum free dim)
        self.kpg = kpg                 # k-chunks (128 nodes each) per adj tile
        self.mpg = mpg                 # m-chunks per xt DMA slice
        self.kc = n // P               # contraction chunks (over all nodes)
        self.nkg = self.kc // kpg      # adj tile groups (= 2 halves x 8 cores)
        self.ni = self.rows // iw      # i-chunks per core
        self.mc = n // P               # stage-1 m-chunks (all nodes)
        self.nmg = self.mc // mpg      # xt DMA slices
        self.mcl = self.rows // P      # stage-3 m-chunks (local rows)
        self.fkc = n_feat // P         # feature contraction chunks
        self.adj_bufs = adj_bufs       # streaming ring depth (512 KiB tiles)
        self.n_cache_kg = n_cache_kg   # kg groups cached in SBUF for pass B
        # fp8 scales (powers of 2, exact): adj x2^sa keeps max < 240;
        # stage-1 operands x2^sx so Delta_fp8 is 2^sx-scaled; pass-B delta
        # x2^sd on device.  psA holds 2^(sa+sx)*h1T', psB 2^(sa+sd)*h2T'.
        self.sa = sa
        self.sd = sd
        self.sx = sx
        assert self.rows % iw == 0 and self.kc % kpg == 0 and self.mc % mpg == 0
        assert self.nkg == 2 * n_cores      # kg = h*8 + cc layout
        assert self.iw % P == 0 and self.ni in (1, 2, 4)
        assert self.n_cache_kg <= n_cores

    # node-chunk index (into Delta / delta_g, natural node order) covered by
    # (kg, kl): kg = h*n_cores + cc covers nodes cc*2048 + h*1024 + kl*128 + p
    def nchunk(self, kg, kl):
        cc, h = kg % self.n_cores, kg // self.n_cores
        return cc * (self.rows // P) + h * self.kpg + kl


# kg groups whose tiles stay resident in SBUF between the passes (32 tiles =
# 16 MiB): most of h=0, plus the last two h=1 groups.  Pass B orders its
# consumption so that while it waits for AG1 it is (a) computing from cache
# and (b) still has streamed-but-AG0-ready groups keeping the DMA busy.
# The whole h=1 half (kg 8..15, x4 strips = 32 tiles = 16 MiB) stays resident
# in SBUF between the passes.  Pass B then opens on STREAMED h0 groups —
# whose delta_g (AG0) completed while pass A was still running — so the DMA
# engines never idle waiting for AG1; the AG1-gated h1 half runs purely from
# cache at the end.  The first CACHE_DEDICATED entries live in the dedicated
# cache buffer; the last two (kg 9, 8 — the cached tiles pass A consumes
# LAST) alias the xt buffer, whose WAR-gated DMAs must not
# head-of-line-block the hw-DMA lanes early on.
CACHED_KG = (15, 14, 13, 12, 11, 10, 9, 8)
CACHE_DEDICATED = 6


def _passA_kg_order(cfg: Cfg):
    """First tile (kg 15) holds stage 1's LAST Delta chunks, serializing
    pass A after stage 1 via real data deps (no slow-mode PE interleave at
    the stage-1/pass-A boundary).  Each pair ENDS on cached groups (11, 10,
    9, 8): they need no ring buffers, so the ring pre-streams pass B's first
    tiles during the pair's tail, shrinking the inter-pass PE gap.  The
    xt-ALIASED groups (kg 9, 8) go very last: their DMAs WAR-wait on stage
    1's xt reads, and issued early they head-of-line-block the hw-DMA
    lanes until stage 1 finishes."""
    return [15, 14, 13, 12, 7, 6, 5, 4, 3, 2, 1, 0, 11, 10, 9, 8]


def _passB_kg_order(cfg: Cfg):
    """Streamed h0 (AG0 already done) first; cached h1 (AG1-gated) groups
    interleave from the point AG1 is typically complete (~6 streamed groups
    in), filling PE gaps in the DMA-paced stretch.  If AG1 is late the PE
    stalls at kg 8 but the DMA ring keeps streaming, so the worst case ties
    the non-interleaved order."""
    return [0, 1, 2, 3, 4, 5, 8, 6, 9, 10, 7, 11, 12, 13, 14, 15]


def build_nc(cfg: Cfg) -> bass.Bass:
    BF = mybir.dt.bfloat16
    F32 = mybir.dt.float32
    FP8 = mybir.dt.float8e4
    n_hid, iw, kpg, fkc = cfg.n_hid, cfg.iw, cfg.kpg, cfg.fkc
    tw = kpg * iw                       # adj tile free width (4096)

    nc = bacc.Bacc("TRN2", target_bir_lowering=False)
    # adjt[kg, pair][p, s*tw + kl*iw + ii] = 2^sa * adjT_shard[node(kg,kl,p),
    # iw*(2*pair+s)+ii] in fp8e4m3 (node(kg,kl,p) per Cfg.nchunk): 1 MiB
    # pair-copies (both strips of a pass-A pair) halve the hw-DMA-lane
    # turnaround overhead vs per-strip 512 KiB copies.
    adjt_h = nc.declare_dram_parameter(
        "adjt4", [cfg.nkg, cfg.ni // 2, P, 2 * tw], FP8, isOutput=False)
    # xt[mg][p, (ml*fkc+k)*128 + c] = fp8(2^sx * x)[128*(mg*mpg+ml)+c, 128*k+p]
    # (stage 1 is replicated: exchanging Delta via collective_compute costs
    # 30-60us for a 1 MiB gather — worse than streaming the full 4 MiB x.
    # x and W1 stream as fp8: their quantization systematics are cancelled
    # exactly by the host-side sidecars, the random part is CLT-damped.)
    xt_h = nc.declare_dram_parameter(
        "xt", [cfg.nmg, P, cfg.mpg * fkc * P], FP8, isOutput=False)
    w1_h = nc.declare_dram_parameter("w1", [fkc, P, n_hid], FP8, isOutput=False)
    b1_h = nc.declare_dram_parameter("b1", [2 * n_hid, 1], F32, isOutput=False)
    w2_h = nc.declare_dram_parameter("w2", [2 * n_hid, 2], F32, isOutput=False)
    # host-side exactness sidecars (see module docstring):
    #   m2  = col-means of the device product bf16(2^sx x)@bf16(W1)  [scaled]
    #   mt  = (true col-means of x@W1 minus fp8(Delta) quantization bias)
    #         * 2^(sa+sx)   -- pass-A correction lhsT
    #   c2/ct = pass-B center estimate (c2 plain, ct * 2^(sa+sd))
    #   rsum  = exact f32 row-sums of this core's adj rows
    c2_h = nc.declare_dram_parameter("c2", [P, 2], F32, isOutput=False)
    ct_h = nc.declare_dram_parameter("ct", [1, 2], F32, isOutput=False)
    # m2 duplicated into both halves for the paired-bank stage-1 evacuation
    m2_h = nc.declare_dram_parameter("m2", [P, 2 * n_hid], F32, isOutput=False)
    mt_h = nc.declare_dram_parameter("mt", [1, n_hid], F32, isOutput=False)
    rs_h = nc.declare_dram_parameter("rsum", [1, cfg.rows], F32, isOutput=False)
    # out[32j + t] = max over i-chunk j (valid for j < ni, t < 2)
    out_h = nc.declare_dram_parameter("out", [P, 1], F32, isOutput=True)

    # collective bounce buffers, one pair per strip-pair:
    # g_in[a][p, 2*m+t] = delta_g_local[a*1024 + 128*m + p, t],  m in [0,8)
    npair = max(1, cfg.ni // 2)
    nstrip = min(2, cfg.ni)
    hmc = cfg.mcl // npair              # local m-chunks per pair (8)
    g_in = [nc.dram_tensor(f"g_in{a}", [P, 2 * hmc], F32)
            for a in range(npair)]
    g_out = [nc.dram_tensor(f"g_out{a}", [P * cfg.n_cores, 2 * hmc], F32,
                            addr_space="Shared") for a in range(npair)]

    seqA = _passA_kg_order(cfg)
    seqB = _passB_kg_order(cfg)

    with tile.TileContext(nc, num_cores=cfg.n_cores) as tc:
        with (
            tc.tile_pool(name="const", bufs=1) as const_pool,
            tc.tile_pool(name="xtp", bufs=1) as xt_pool,
            tc.tile_pool(name="xw1p", bufs=1) as xw1_pool,
            tc.tile_pool(name="h1tp", bufs=1) as h1t_pool,
            tc.tile_pool(name="cachep", bufs=1) as cache_pool,
            tc.tile_pool(name="adjp", bufs=cfg.adj_bufs) as adj_pool,
            tc.tile_pool(name="gp", bufs=1) as g_pool,
            tc.tile_pool(name="mxp", bufs=1) as mx_pool,
            tc.tile_pool(name="ps1p", bufs=3, space="PSUM") as ps1_pool,
            tc.tile_pool(name="psAp", bufs=1, space="PSUM") as psA_pool,
            tc.tile_pool(name="ps3p", bufs=2, space="PSUM") as ps3_pool,
            tc.tile_pool(name="psBp", bufs=1, space="PSUM") as psB_pool,
        ):
            # ---- constants first: every stage-1 matmul needs w1, so it must
            # land on a lane AHEAD of the xt flood (lanes execute in order).
            w1_sb = const_pool.tile([P, fkc * n_hid], FP8)
            for k in range(fkc):
                nc.sync.dma_start(
                    out=w1_sb[:, k * n_hid:(k + 1) * n_hid], in_=w1_h[k])
            b1_sb = const_pool.tile([2 * n_hid, 1], F32)
            nc.sync.dma_start(out=b1_sb[:, :], in_=b1_h[:, :])
            w2_sb = const_pool.tile([2 * n_hid, 2], F32)
            nc.sync.dma_start(out=w2_sb[:, :], in_=w2_h[:, :])
            c2_sb = const_pool.tile([P, 2], F32)
            nc.sync.dma_start(out=c2_sb[:, :], in_=c2_h[:, :])
            ct_sb = const_pool.tile([1, 2], F32)
            nc.sync.dma_start(out=ct_sb[:, :], in_=ct_h[:, :])
            m2_sb = const_pool.tile([P, 2 * n_hid], F32)
            nc.sync.dma_start(out=m2_sb[:, :], in_=m2_h[:, :])
            mt_sb = const_pool.tile([1, n_hid], F32)
            nc.sync.dma_start(out=mt_sb[:, :], in_=mt_h[:, :])
            rs_sb = const_pool.tile([1, cfg.rows], F32)
            nc.sync.dma_start(out=rs_sb[:, :], in_=rs_h[:, :])

            # ---- xt next: 2 partition-split copies per slice (32 total) so
            # slice 0 lands on two hw-DMA lanes in parallel and stage 1
            # starts long before the full 4 MiB is in.  All 32 copies sit
            # ahead of the adj flood in the lane rotation.
            xt_sb = xt_pool.tile([P, cfg.nmg * cfg.mpg * fkc * P], FP8)
            xg = cfg.mpg * fkc * P
            for mg in range(cfg.nmg):
                for ph in range(2):
                    nc.sync.dma_start(
                        out=xt_sb[64 * ph:64 * (ph + 1),
                                  mg * xg:(mg + 1) * xg],
                        in_=xt_h[mg][64 * ph:64 * (ph + 1), :])

            # ---- stage 1: Delta = (2^sx x)@W1 - m2, stored fp8 node-major.
            # DoubleRow contracts both 128-feature chunks in one instruction
            # (fp8 fast path); two m-chunks share one psum bank so a single
            # double-width DVE subtract evacuates both (the sub chain would
            # otherwise pace stage 1).
            DR = mybir.MatmulPerfMode.DoubleRow
            xw1_sb = xw1_pool.tile([P, cfg.mc * n_hid], FP8)
            for mp in range(cfg.mc // 2):
                ps1 = ps1_pool.tile([P, 2 * n_hid], F32, tag="ps1")
                for j in range(2):
                    m = 2 * mp + j
                    nc.tensor.matmul(
                        ps1[:, j * n_hid:(j + 1) * n_hid],
                        lhsT=xt_sb[:, m * fkc * P:(m + 1) * fkc * P].rearrange(
                            "p (two f) -> p two f", two=2),
                        rhs=w1_sb[:, :].rearrange("p (two f) -> p two f", two=2),
                        start=True, stop=True, perf_mode=DR,
                    )
                nc.vector.tensor_sub(
                    xw1_sb[:, 2 * mp * n_hid:2 * (mp + 1) * n_hid], ps1[:, :],
                    m2_sb[:, :])

            # ---- SBUF cache for adj tiles reused by pass B (CACHED_KG x 4
            # strips = 32 tiles = 16 MiB): 24 in a dedicated buffer, 8
            # aliasing the xt buffer (dead after stage 1; the framework
            # WAR-orders each aliased cache DMA after stage 1's last read of
            # the overlapping columns).
            ncd = CACHE_DEDICATED
            cache_sb = cache_pool.tile([P, cfg.ni * ncd * tw], FP8)
            cache_idx = {kg: i for i, kg in enumerate(CACHED_KG)}

            def adj_tile_slice(n_i, kg, c0, c1):
                """AP for fp8 columns [c0:c1) of strip n_i within the cached
                1 MiB pair-unit (kg, n_i//2)."""
                u = 2 * cache_idx[kg] + n_i // 2
                base = (n_i % 2) * tw
                if u < 2 * ncd:
                    off = u * 2 * tw + base
                    return cache_sb[:, off + c0:off + c1]
                off = (u - 2 * ncd) * 2 * tw + base
                return xt_sb[:, off + c0:off + c1]

            # ---- pass A: 2^(sa+sx) h1T' = Delta.T @ adjT_fp8 + mt.T @ rsum
            # h1t[64s + h, a*iw + ii] = h1 for i-chunk (2a+s) (strip s in
            # array columns [64s, 64s+64), both strips share one psum bank)
            h1t_sb = h1t_pool.tile([nstrip * n_hid, npair * iw], F32)
            gl_sb = g_pool.tile([P, 2 * cfg.mcl], F32)
            gf_sb = [g_pool.tile([P, 2 * cfg.n_cores * hmc], F32,
                                 name=f"gf_sb{a}") for a in range(npair)]
            g_sb = [g_pool.tile([P, 2 * cfg.n_cores * hmc], FP8,
                                name=f"g_sb{a}") for a in range(npair)]
            for a in range(npair):
                # one psum bank per strip (partition 0): DoubleRow weights
                # occupy 2M=128 array columns, so the two strips cannot be
                # column-packed into one bank via tile_position.
                psA = [psA_pool.tile([n_hid, iw], F32, tag=f"psA{s}",
                                     name=f"psA{s}") for s in range(nstrip)]
                for idx, kg in enumerate(seqA):
                    # one 1 MiB pair-copy per (kg, pair): both strips' tiles
                    if kg in cache_idx:
                        nc.sync.dma_start(
                            out=adj_tile_slice(2 * a, kg, 0, 2 * tw),
                            in_=adjt_h[kg, a])
                        rhss = [
                            lambda c0, c1, n_i=nstrip * a + s, kg=kg:
                            adj_tile_slice(n_i, kg, c0, c1)
                            for s in range(nstrip)]
                    else:
                        at = adj_pool.tile([P, 2 * tw], FP8, tag="at")
                        nc.sync.dma_start(out=at[:, :], in_=adjt_h[kg, a])
                        rhss = [
                            lambda c0, c1, at=at, s=s:
                            at[:, s * tw + c0:s * tw + c1]
                            for s in range(nstrip)]
                    for kl in range(0, kpg, 2):
                        # DoubleRow: chunks (kg,kl) and (kg,kl+1) in one
                        # instruction — consecutive kl = consecutive node
                        # chunks, so both Delta and the adj tile are already
                        # plane-major-contiguous.
                        k = cfg.nchunk(kg, kl)
                        for s in range(nstrip):
                            nc.tensor.matmul(
                                psA[s][:, :],
                                lhsT=xw1_sb[:, k * n_hid:(k + 2) * n_hid]
                                .rearrange("p (two f) -> p two f", two=2),
                                rhs=rhss[s](kl * iw, (kl + 2) * iw)
                                .rearrange("p (two f) -> p two f", two=2),
                                start=(idx == 0 and kl == 0), stop=False,
                                perf_mode=DR,
                            )
                for s in range(nstrip):
                    nc.tensor.matmul(
                        psA[s][:, :],
                        lhsT=mt_sb[:, :],
                        rhs=rs_sb[:, (nstrip * a + s) * iw:(nstrip * a + s + 1) * iw],
                        start=False, stop=True,
                    )
                    # h1 = relu(2^-(sa+sx) * psA + b1), exact descale in fp32
                    nc.scalar.activation(
                        h1t_sb[s * n_hid:(s + 1) * n_hid,
                               a * iw:(a + 1) * iw], psA[s][:, :],
                        mybir.ActivationFunctionType.Relu,
                        bias=b1_sb[:n_hid, :],
                        scale=float(2.0 ** -(cfg.sa + cfg.sx)),
                    )
                # ---- stage 3 for this pair: delta_g = h1 @ W2 - c (fp32)
                for s in range(nstrip):
                    for ml in range(iw // P):
                        m = (nstrip * a + s) * (iw // P) + ml
                        ps3 = ps3_pool.tile([P, 2], F32, tag="ps3")
                        nc.tensor.matmul(
                            ps3[:, :],
                            lhsT=h1t_sb[s * n_hid:(s + 1) * n_hid,
                                        a * iw + ml * P:a * iw + (ml + 1) * P],
                            rhs=w2_sb[s * n_hid:(s + 1) * n_hid, :],
                            start=True, stop=True,
                        )
                        nc.vector.tensor_sub(
                            gl_sb[:, 2 * m:2 * m + 2], ps3[:, :], c2_sb[:, :])
                # ---- AllGather this pair's delta_g.  g_in rides the SWDGE
                # (Pool-engine) path: the HWDGE lanes serialize round-robin
                # with the adj stream, which would delay this tiny copy ~12us.
                nc.gpsimd.dma_start(
                    out=g_in[a][:, :],
                    in_=gl_sb[:, 2 * a * hmc:2 * (a + 1) * hmc])
                nc.gpsimd.collective_compute(
                    "AllGather", mybir.AluOpType.bypass,
                    ins=[g_in[a][:, :]], outs=[g_out[a][:, :]],
                    replica_groups=[list(range(cfg.n_cores))],
                )
                # g_out[a][(r*128+p), 2*m+t] -> gf[a][p, (r*hmc+m)*2+t]
                # SWDGE again: on a HWDGE lane this copy's Collectives wait
                # would head-of-line-block the pass-B adj prefetch behind it.
                nc.gpsimd.dma_start(
                    out=gf_sb[a][:, :].rearrange(
                        "p (r c) -> p r c", r=cfg.n_cores),
                    in_=g_out[a][:, :].rearrange("(r p) c -> p r c", p=P))

            # fp8 converts AFTER the pair loop: the scalar queue is in-order,
            # and convert-0 (gated on AG0) emitted before relu-1 would block
            # relu-1 -> stage-3 -> AG1 by several us.
            for a in range(npair):
                nc.scalar.activation(
                    g_sb[a][:, :], gf_sb[a][:, :],
                    mybir.ActivationFunctionType.Copy,
                    scale=float(2 ** cfg.sd))

            # ---- pass B: all ni i-chunks packed into ONE [128, iw] psum bank
            # via PE column-tiling: strip j (array cols [32j, 32j+32)) computes
            # i-chunk j.  2^(sa+sd) h2T'[t, i] lands at psum[32j + t, ii].
            # lhsT for chunk (kg, kl): g_sb[h][:, 2*(cc*kpg+kl) : +2].
            psB = psB_pool.tile([P, iw], F32)
            for idx, kg in enumerate(seqB):
                cc, h = kg % cfg.n_cores, kg // cfg.n_cores
                rhss = []
                if kg in cache_idx:                      # cached from pass A
                    for n_i in range(cfg.ni):
                        rhss.append(
                            lambda c0, c1, n_i=n_i, kg=kg:
                            adj_tile_slice(n_i, kg, c0, c1))
                else:
                    ats = []
                    for pr in range(cfg.ni // 2):
                        at = adj_pool.tile([P, 2 * tw], FP8, tag="at")
                        nc.sync.dma_start(out=at[:, :], in_=adjt_h[kg, pr])
                        ats.append(at)
                    for n_i in range(cfg.ni):
                        rhss.append(
                            lambda c0, c1, at=ats[n_i // 2], s=n_i % 2:
                            at[:, s * tw + c0:s * tw + c1])
                for kl in range(kpg):
                    gcol = 2 * (cc * kpg + kl)
                    for n_i in range(cfg.ni):
                        nc.tensor.matmul(
                            psB[32 * n_i:32 * n_i + 2, :],
                            lhsT=g_sb[h][:, gcol:gcol + 2],
                            rhs=rhss[n_i](kl * iw, (kl + 1) * iw),
                            start=(idx == 0 and kl == 0), stop=False,
                            tile_position=(0, 32 * n_i),
                            skip_group_check=True,
                        )
            for n_i in range(cfg.ni):
                nc.tensor.matmul(
                    psB[32 * n_i:32 * n_i + 2, :],
                    lhsT=ct_sb[:, :],
                    rhs=rs_sb[:, n_i * iw:(n_i + 1) * iw],
                    start=False, stop=True,
                    tile_position=(0, 32 * n_i),
                    skip_group_check=True,
                )
            # per-partition max over the free axis in ONE reduce (partitions
            # are independent; the host only reads rows 32j + t, the rest is
            # harmless junk from unwritten psum partitions)
            mxsb = mx_pool.tile([P, 1], F32)
            nc.vector.reduce_max(
                mxsb[:, :], psB[:, :], axis=mybir.AxisListType.X)
            mxo = mx_pool.tile([P, 1], F32)
            nc.scalar.mul(mxo[:, :], mxsb[:, :], float(2.0 ** -(cfg.sa + cfg.sd)))
            nc.sync.dma_start(out=out_h[:, :], in_=mxo[:, :])
    nc.compile()
    return nc


def shard_inputs(cfg: Cfg, x, adj, W1, b1, W2):
    """Host-side prep: pre-tile + quantize, and build the exactness sidecars
    (see module docstring)."""
    x = np.asarray(x, dtype=np.float32)
    adj = np.asarray(adj, dtype=np.float32)

    sxf = np.float32(2.0 ** cfg.sx)
    # xt[mg, p, ml, k, c] = fp8(2^sx * x)[128*(mg*mpg+ml)+c, 128*k+p]
    xb = (x * sxf).astype(FP8_NP)
    assert np.isfinite(xb.astype(np.float32)).all()
    xt = xb.reshape(cfg.nmg, cfg.mpg, P, cfg.fkc, P).transpose(0, 4, 1, 3, 2)
    xt = np.ascontiguousarray(xt).reshape(cfg.nmg, P, cfg.mpg * cfg.fkc * P)

    W1f = np.asarray(W1, dtype=np.float32)
    b1f = np.asarray(b1, dtype=np.float32)
    W2f = np.asarray(W2, dtype=np.float32)
    w1b = W1f.astype(FP8_NP)
    w1 = np.ascontiguousarray(w1b.reshape(cfg.fkc, P, cfg.n_hid))
    # b1/W2 duplicated into both partition halves for the pass-A 2x packing
    b1d = np.ascontiguousarray(
        np.concatenate([b1f, b1f]).reshape(2 * cfg.n_hid, 1))
    w2 = np.ascontiguousarray(np.vstack([W2f, W2f]))

    # --- pass-A sidecars: exact simulation of the device quantizations.
    # device stage-1 product (2^sx-scaled), bf16 operands, f32 accumulate:
    xW1_dev = xb.astype(np.float32) @ w1b.astype(np.float32)     # 2^sx-scaled
    m_dev = xW1_dev.mean(axis=0, dtype=np.float64).astype(np.float32)
    Q = xW1_dev - m_dev                                          # device Delta
    Qq = Q.astype(FP8_NP).astype(np.float32)                     # fp8(Delta)
    assert np.isfinite(Qq).all(), "Delta overflows fp8 range"
    eps = (Qq - Q).mean(axis=0, dtype=np.float64).astype(np.float32)
    m_true = (x.mean(axis=0, dtype=np.float64).astype(np.float32) @ W1f)
    # correction lhsT: in 2^(sa+sx)-scaled psum units per unit rowsum
    mt_val = (m_true * sxf - eps) * np.float32(2.0 ** cfg.sa)
    m2 = np.ascontiguousarray(np.broadcast_to(
        np.tile(m_dev, 2), (P, 2 * cfg.n_hid)).astype(np.float32))
    mt = np.ascontiguousarray(mt_val.reshape(1, cfg.n_hid).astype(np.float32))

    # --- pass-B center estimate from a row subsample (any c is exact;
    # closer c => smaller |delta_g| => less fp8 noise)
    idx = np.arange(0, cfg.n, max(1, cfg.n // 256))
    g_sub = np.maximum(adj[idx] @ (xW1_dev / sxf) + b1f, 0.0) @ W2f
    c_est = g_sub.mean(axis=0).astype(np.float32)                # [2]
    c2 = np.ascontiguousarray(np.broadcast_to(c_est, (P, 2)).astype(np.float32))
    ct = np.ascontiguousarray(
        (c_est * np.float32(2.0 ** (cfg.sa + cfg.sd))).reshape(1, 2))
    rsum = adj.sum(axis=1, dtype=np.float64).astype(np.float32)  # [n]

    saf = np.float32(2.0 ** cfg.sa)
    in_maps = []
    for c in range(cfg.n_cores):
        shard = adj[c * cfg.rows:(c + 1) * cfg.rows, :]
        # a[n_i, kg=(h, cc), p, kl, ii] = shard[iw*n_i+ii, node(kg,kl,p)]
        # node(kg,kl,p) = cc*2048 + h*1024 + kl*128 + p
        a6 = shard.reshape(cfg.ni, cfg.iw,
                           cfg.n_cores, 2, cfg.kpg, P)   # [ni,ii,cc,h,kl,p]
        a6 = a6.transpose(0, 3, 2, 5, 4, 1)              # [ni,h,cc,p,kl,ii]
        a2 = np.ascontiguousarray((a6 * saf).astype(FP8_NP)).reshape(
            cfg.ni, cfg.nkg, P, cfg.kpg * cfg.iw)
        # 1 MiB pair-copies: [kg, pair, P, s*tw + col], n_i = 2*pair + s
        tw = cfg.kpg * cfg.iw
        a2 = a2.reshape(cfg.ni // 2, 2, cfg.nkg, P, tw).transpose(
            2, 0, 3, 1, 4)
        a2 = np.ascontiguousarray(a2).reshape(cfg.nkg, cfg.ni // 2, P, 2 * tw)
        rs = np.ascontiguousarray(
            rsum[c * cfg.rows:(c + 1) * cfg.rows].reshape(1, cfg.rows))
        in_maps.append({"adjt4": a2, "xt": xt, "w1": w1, "b1": b1d,
                        "w2": w2, "c2": c2, "ct": ct, "m2": m2, "mt": mt,
                        "rsum": rs})
    return in_maps


def finish_on_host(cfg: Cfg, per_core_out, b2, W3, b3):
    """per_core_out: [n_cores, 128] device outputs (strip j's maxima at
    [32j + t]) -> [1,1,1] final output."""
    b2 = np.asarray(b2, dtype=np.float32)
    W3 = np.asarray(W3, dtype=np.float32)
    b3 = np.asarray(b3, dtype=np.float32)
    strips = np.stack([per_core_out[:, 32 * j:32 * j + 2]
                       for j in range(cfg.ni)])          # [ni, n_cores, 2]
    pooled = strips.max(axis=(0, 1)).astype(np.float32) + b2       # [2]
    out = pooled[None, None, :] @ W3.T + b3                        # [1,1,1]
    return out.astype(np.float32)


_NC_CACHE: dict = {}
LAST_RESULT = None  # BassKernelResults of the most recent run (for test.py)


def kernel(x, adj, W1, b1, W2, b2, W3, b3):
    cfg = Cfg()
    x = np.asarray(x)
    assert x.shape == (cfg.n, cfg.n_feat), x.shape
    if "nc" not in _NC_CACHE:
        _NC_CACHE["nc"] = build_nc(cfg)
    nc = _NC_CACHE["nc"]

    in_maps = shard_inputs(cfg, x, adj, W1, b1, W2)
    trace = os.environ.get("GCN_TRACE", "0") == "1"
    res = run_bass_kernel_spmd(
        nc, in_maps, core_ids=list(range(cfg.n_cores)), trace=trace)
    global LAST_RESULT
    LAST_RESULT = res
    per_core = np.stack(
        [np.asarray(r["out"][:, 0], dtype=np.float32) for r in res.results])
    return finish_on_host(cfg, per_core, b2, W3, b3)


# revision 68
# speedup vs baseline: 1.1118x; 1.1118x over previous
"""GCN forward (2x graph-conv + global max-pool + linear) on 8 TRN2 NeuronCores.

Reference computation (N=16384 nodes, 256 feats, 64 hid):
    h1 = relu(adj @ (x @ W1) + b1)          [N, 64]
    h2 = adj @ (h1 @ W2) + b2               [N, 2]
    out = max(h2, axis=0) @ W3.T + b3       [1, 1, 1]

Distribution: row-shard adj over the 8 cores (core c owns output rows
[c*2048, (c+1)*2048)).  Each core:
  stage 1: Delta = bf16(16x)@bf16(W1) - m2, stored fp8  (replicated, tiny)
  pass A : h1T' = Delta.T @ adjT_fp8 + mt.T@rsum        [64, 2048] scaled
           bias/relu fused on psum evacuation (exact descale via act scale)
  stage 3: g_local = h1 @ W2 (fp32); delta_g = g_local - c
  AllGather delta_g (split per strip-pair) -> on-device collective
  pass B : h2T' = delta_g.T @ adjT_fp8 + ct.T@rsum      4x column-packed in
           one PSUM bank via tile_position; per-i-chunk max -> [128, 1]
Host: unpack/max over strips and cores, + b2, @ W3.T + b3.

Perf structure (~224-238us vs the 305us baseline; run-to-run spread is
cross-core collective skew; see comments inline):
  - adj streams as fp8e4m3 in 512 KiB tiles; k-chunks are grouped
    kg = h*8 + cc (cc = owning core, h = half) so that strip-pair `a`
    of every core produces exactly the h=a delta_g chunks -> the
    AllGather is split per pair; AG0 completes under pass-A pair 1.
  - x and W1 also stream as fp8 (the sidecars cancel the quantization
    systematics exactly); stage 1 and pass A use DoubleRow matmuls
    (fp8 fast path, 256-deep contraction per instruction).
  - pass A consumes kg groups so its first matmul data-depends on
    stage 1's LAST output chunk: stage 1 (fp8 M=128) and pass A never
    interleave on the PE (interleaving different matmul flavors drops
    the fp8 double-pump rate ~3x).  Each pair ends on CACHED groups so
    the DMA ring pre-streams pass B's first tiles during the tail.
  - the whole h=1 adj half (32 tiles = 16 MiB) stays resident in SBUF
    between the passes (24 tiles dedicated + 8 aliasing the dead xt
    buffer), cutting the 64 MiB/core adj stream to ~52 MiB.  Pass B
    opens on STREAMED h0 groups (AG0 long done) so the DMA engines
    never idle while AG1 completes; the AG1-gated h1 half runs purely
    from cache at the end.
  - HW-DGE copies are serialized round-robin onto 8 semaphore lanes;
    every latency-critical small copy (g / gf bounces) rides the SWDGE
    (Pool-engine) path instead so it is never queued behind the adj
    stream, and consts are issued before xt before adj so each lane
    serves them in dependency order.

fp8 noise is harmless because both passes compute the large mean
component exactly in fp32 via host-side sidecars:
  - rsum: exact f32 row-sums of adj (the only O(N^2) host work),
  - mt/ct: column-means, with exact cancellation of every quantization
    systematic (host simulates the device's bf16/fp8 quantization
    exactly); only sqrt(N)-damped random noise survives.
"""

import os
import sys

sys.path.insert(0, "/opt/trn_rl_repo")

import numpy as np
import ml_dtypes


def _install_ntff_hook_shim():
    """The image's `antenv` lacks `axon_hooks`, which bass_utils imports for
    trace=True under axon. Provide it, wired to the PJRT .so's NRT-profile
    C ABI (same thing trn_boot would have registered)."""
    import types
    if "antenv.axon_hooks" in sys.modules:
        return
    try:
        import antenv  # noqa: F401
        from trn_agent_boot.trn_boot import _ntff_profile_via_ctypes
        mod = types.ModuleType("antenv.axon_hooks")
        _state = {"hook": _ntff_profile_via_ctypes("/opt/axon/libaxon_pjrt.so")}
        mod.set_axon_ntff_profile_hook = lambda h: _state.update(hook=h)
        mod.get_axon_ntff_profile_hook = lambda: _state["hook"]
        sys.modules["antenv.axon_hooks"] = mod
    except Exception:
        pass


_install_ntff_hook_shim()

import concourse.bass as bass
import concourse.mybir as mybir
import concourse.tile as tile
from concourse import bacc
from concourse.bass_utils import run_bass_kernel_spmd

BF16_NP = ml_dtypes.bfloat16
FP8_NP = ml_dtypes.float8_e4m3

P = 128          # partition dim
N_CORES = 8
N_NODES = 16384
N_FEAT = 256
N_HID = 64


class Cfg:
    def __init__(self, n=N_NODES, n_feat=N_FEAT, n_hid=N_HID, n_cores=N_CORES,
                 iw=512, kpg=8, mpg=8, adj_bufs=12, n_cache_kg=4,
                 sa=21, sd=10, sx=4):
        self.n, self.n_feat, self.n_hid, self.n_cores = n, n_feat, n_hid, n_cores
        self.rows = n // n_cores       # output rows per core
        self.iw = iw                   # i-tile width (psum free dim)
        self.kpg = kpg                 # k-chunks (128 nodes each) per adj tile
        self.mpg = mpg                 # m-chunks per xt DMA slice
        self.kc = n // P               # contraction chunks (over all nodes)
        self.nkg = self.kc // kpg      # adj tile groups (= 2 halves x 8 cores)
        self.ni = self.rows // iw      # i-chunks per core
        self.mc = n // P               # stage-1 m-chunks (all nodes)
        self.nmg = self.mc // mpg      # xt DMA slices
        self.mcl = self.rows // P      # stage-3 m-chunks (local rows)
        self.fkc = n_feat // P         # feature contraction chunks
        self.adj_bufs = adj_bufs       # streaming ring depth (512 KiB tiles)
        self.n_cache_kg = n_cache_kg   # kg groups cached in SBUF for pass B
        # fp8 scales (powers of 2, exact): adj x2^sa keeps max < 240;
        # stage-1 operands x2^sx so Delta_fp8 is 2^sx-scaled; pass-B delta
        # x2^sd on device.  psA holds 2^(sa+sx)*h1T', psB 2^(sa+sd)*h2T'.
        self.sa = sa
        self.sd = sd
        self.sx = sx
        assert self.rows % iw == 0 and self.kc % kpg == 0 and self.mc % mpg == 0
        assert self.nkg == 2 * n_cores      # kg = h*8 + cc layout
        assert self.iw % P == 0 and self.ni in (1, 2, 4)
        assert self.n_cache_kg <= n_cores

    # node-chunk index (into Delta / delta_g, natural node order) covered by
    # (kg, kl): kg = h*n_cores + cc covers nodes cc*2048 + h*1024 + kl*128 + p
    def nchunk(self, kg, kl):
        cc, h = kg % self.n_cores, kg // self.n_cores
        return cc * (self.rows // P) + h * self.kpg + kl


# kg groups whose tiles stay resident in SBUF between the passes (32 tiles =
# 16 MiB): most of h=0, plus the last two h=1 groups.  Pass B orders its
# consumption so that while it waits for AG1 it is (a) computing from cache
# and (b) still has streamed-but-AG0-ready groups keeping the DMA busy.
# The whole h=1 half (kg 8..15, x4 strips = 32 tiles = 16 MiB) stays resident
# in SBUF between the passes.  Pass B then opens on STREAMED h0 groups —
# whose delta_g (AG0) completed while pass A was still running — so the DMA
# engines never idle waiting for AG1; the AG1-gated h1 half runs purely from
# cache at the end.  The first CACHE_DEDICATED entries live in the dedicated
# cache buffer; the last two (kg 9, 8 — the cached tiles pass A consumes
# LAST) alias the xt buffer, whose WAR-gated DMAs must not
# head-of-line-block the hw-DMA lanes early on.
CACHED_KG = (15, 14, 13, 12, 11, 10, 9, 8)
CACHE_DEDICATED = 6


def _passA_kg_order(cfg: Cfg):
    """First tile (kg 15) holds stage 1's LAST Delta chunks, serializing
    pass A after stage 1 via real data deps (no slow-mode PE interleave at
    the stage-1/pass-A boundary).  Each pair ENDS on cached groups (11, 10,
    9, 8): they need no ring buffers, so the ring pre-streams pass B's first
    tiles during the pair's tail, shrinking the inter-pass PE gap.  The
    xt-ALIASED groups (kg 9, 8) go very last: their DMAs WAR-wait on stage
    1's xt reads, and issued early they head-of-line-block the hw-DMA
    lanes until stage 1 finishes."""
    return [15, 14, 13, 12, 7, 6, 5, 4, 3, 2, 1, 0, 11, 10, 9, 8]


def _passB_kg_order(cfg: Cfg):
    """Streamed h0 (AG0 already done) first; cached h1 (AG1-gated) groups
    interleave from the point AG1 is typically complete (~6 streamed groups
    in), filling PE gaps in the DMA-paced stretch.  If AG1 is late the PE
    stalls at kg 8 but the DMA ring keeps streaming, so the worst case ties
    the non-interleaved order."""
    return [0, 1, 2, 3, 4, 5, 8, 6, 9, 10, 7, 11, 12, 13, 14, 15]


def build_nc(cfg: Cfg) -> bass.Bass:
    BF = mybir.dt.bfloat16
    F32 = mybir.dt.float32
    FP8 = mybir.dt.float8e4
    n_hid, iw, kpg, fkc = cfg.n_hid, cfg.iw, cfg.kpg, cfg.fkc
    tw = kpg * iw                       # adj tile free width (4096)

    nc = bacc.Bacc("TRN2", target_bir_lowering=False)
    # adjt[n_i, kg][p, kl*iw + ii] = 2^sa * adjT_shard[node(kg,kl,p),
    # iw*n_i+ii] in fp8e4m3 (node(kg,kl,p) per Cfg.nchunk).
    adjt_h = nc.declare_dram_parameter(
        "adjt3", [cfg.ni, cfg.nkg, P, tw], FP8, isOutput=False)
    # xt[mg][p, (ml*fkc+k)*128 + c] = fp8(2^sx * x)[128*(mg*mpg+ml)+c, 128*k+p]
    # (stage 1 is replicated: exchanging Delta via collective_compute costs
    # 30-60us for a 1 MiB gather — worse than streaming the full 4 MiB x.
    # x and W1 stream as fp8: their quantization systematics are cancelled
    # exactly by the host-side sidecars, the random part is CLT-damped.)
    xt_h = nc.declare_dram_parameter(
        "xt", [cfg.nmg, P, cfg.mpg * fkc * P], FP8, isOutput=False)
    w1_h = nc.declare_dram_parameter("w1", [fkc, P, n_hid], FP8, isOutput=False)
    b1_h = nc.declare_dram_parameter("b1", [2 * n_hid, 1], F32, isOutput=False)
    w2_h = nc.declare_dram_parameter("w2", [2 * n_hid, 2], F32, isOutput=False)
    # host-side exactness sidecars (see module docstring):
    #   m2  = col-means of the device product bf16(2^sx x)@bf16(W1)  [scaled]
    #   mt  = (true col-means of x@W1 minus fp8(Delta) quantization bias)
    #         * 2^(sa+sx)   -- pass-A correction lhsT
    #   c2/ct = pass-B center estimate (c2 plain, ct * 2^(sa+sd))
    #   rsum  = exact f32 row-sums of this core's adj rows
    c2_h = nc.declare_dram_parameter("c2", [P, 2], F32, isOutput=False)
    ct_h = nc.declare_dram_parameter("ct", [1, 2], F32, isOutput=False)
    # m2 duplicated into both halves for the paired-bank stage-1 evacuation
    m2_h = nc.declare_dram_parameter("m2", [P, 2 * n_hid], F32, isOutput=False)
    mt_h = nc.declare_dram_parameter("mt", [1, n_hid], F32, isOutput=False)
    rs_h = nc.declare_dram_parameter("rsum", [1, cfg.rows], F32, isOutput=False)
    # out[32j + t] = max over i-chunk j (valid for j < ni, t < 2)
    out_h = nc.declare_dram_parameter("out", [P, 1], F32, isOutput=True)

    # collective bounce buffers, one pair per strip-pair:
    # g_in[a][p, 2*m+t] = delta_g_local[a*1024 + 128*m + p, t],  m in [0,8)
    npair = max(1, cfg.ni // 2)
    nstrip = min(2, cfg.ni)
    hmc = cfg.mcl // npair              # local m-chunks per pair (8)
    g_in = [nc.dram_tensor(f"g_in{a}", [P, 2 * hmc], F32)
            for a in range(npair)]
    g_out = [nc.dram_tensor(f"g_out{a}", [P * cfg.n_cores, 2 * hmc], F32,
                            addr_space="Shared") for a in range(npair)]

    seqA = _passA_kg_order(cfg)
    seqB = _passB_kg_order(cfg)

    with tile.TileContext(nc, num_cores=cfg.n_cores) as tc:
        with (
            tc.tile_pool(name="const", bufs=1) as const_pool,
            tc.tile_pool(name="xtp", bufs=1) as xt_pool,
            tc.tile_pool(name="xw1p", bufs=1) as xw1_pool,
            tc.tile_pool(name="h1tp", bufs=1) as h1t_pool,
            tc.tile_pool(name="cachep", bufs=1) as cache_pool,
            tc.tile_pool(name="adjp", bufs=cfg.adj_bufs) as adj_pool,
            tc.tile_pool(name="gp", bufs=1) as g_pool,
            tc.tile_pool(name="mxp", bufs=1) as mx_pool,
            tc.tile_pool(name="ps1p", bufs=3, space="PSUM") as ps1_pool,
            tc.tile_pool(name="psAp", bufs=1, space="PSUM") as psA_pool,
            tc.tile_pool(name="ps3p", bufs=2, space="PSUM") as ps3_pool,
            tc.tile_pool(name="psBp", bufs=1, space="PSUM") as psB_pool,
        ):
            # ---- constants first: every stage-1 matmul needs w1, so it must
            # land on a lane AHEAD of the xt flood (lanes execute in order).
            w1_sb = const_pool.tile([P, fkc * n_hid], FP8)
            for k in range(fkc):
                nc.sync.dma_start(
                    out=w1_sb[:, k * n_hid:(k + 1) * n_hid], in_=w1_h[k])
            b1_sb = const_pool.tile([2 * n_hid, 1], F32)
            nc.sync.dma_start(out=b1_sb[:, :], in_=b1_h[:, :])
            w2_sb = const_pool.tile([2 * n_hid, 2], F32)
            nc.sync.dma_start(out=w2_sb[:, :], in_=w2_h[:, :])
            c2_sb = const_pool.tile([P, 2], F32)
            nc.sync.dma_start(out=c2_sb[:, :], in_=c2_h[:, :])
            ct_sb = const_pool.tile([1, 2], F32)
            nc.sync.dma_start(out=ct_sb[:, :], in_=ct_h[:, :])
            m2_sb = const_pool.tile([P, 2 * n_hid], F32)
            nc.sync.dma_start(out=m2_sb[:, :], in_=m2_h[:, :])
            mt_sb = const_pool.tile([1, n_hid], F32)
            nc.sync.dma_start(out=mt_sb[:, :], in_=mt_h[:, :])
            rs_sb = const_pool.tile([1, cfg.rows], F32)
            nc.sync.dma_start(out=rs_sb[:, :], in_=rs_h[:, :])

            # ---- xt next: 2 partition-split copies per slice (32 total) so
            # slice 0 lands on two hw-DMA lanes in parallel and stage 1
            # starts long before the full 4 MiB is in.  All 32 copies sit
            # ahead of the adj flood in the lane rotation.
            xt_sb = xt_pool.tile([P, cfg.nmg * cfg.mpg * fkc * P], FP8)
            xg = cfg.mpg * fkc * P
            for mg in range(cfg.nmg):
                for ph in range(2):
                    nc.sync.dma_start(
                        out=xt_sb[64 * ph:64 * (ph + 1),
                                  mg * xg:(mg + 1) * xg],
                        in_=xt_h[mg][64 * ph:64 * (ph + 1), :])

            # ---- stage 1: Delta = (2^sx x)@W1 - m2, stored fp8 node-major.
            # DoubleRow contracts both 128-feature chunks in one instruction
            # (fp8 fast path); two m-chunks share one psum bank so a single
            # double-width DVE subtract evacuates both (the sub chain would
            # otherwise pace stage 1).
            DR = mybir.MatmulPerfMode.DoubleRow
            xw1_sb = xw1_pool.tile([P, cfg.mc * n_hid], FP8)
            for mp in range(cfg.mc // 2):
                ps1 = ps1_pool.tile([P, 2 * n_hid], F32, tag="ps1")
                for j in range(2):
                    m = 2 * mp + j
                    nc.tensor.matmul(
                        ps1[:, j * n_hid:(j + 1) * n_hid],
                        lhsT=xt_sb[:, m * fkc * P:(m + 1) * fkc * P].rearrange(
                            "p (two f) -> p two f", two=2),
                        rhs=w1_sb[:, :].rearrange("p (two f) -> p two f", two=2),
                        start=True, stop=True, perf_mode=DR,
                    )
                nc.vector.tensor_sub(
                    xw1_sb[:, 2 * mp * n_hid:2 * (mp + 1) * n_hid], ps1[:, :],
                    m2_sb[:, :])

            # ---- SBUF cache for adj tiles reused by pass B (CACHED_KG x 4
            # strips = 32 tiles = 16 MiB): 24 in a dedicated buffer, 8
            # aliasing the xt buffer (dead after stage 1; the framework
            # WAR-orders each aliased cache DMA after stage 1's last read of
            # the overlapping columns).
            ncd = CACHE_DEDICATED
            cache_sb = cache_pool.tile([P, cfg.ni * ncd * tw], FP8)
            cache_idx = {kg: i for i, kg in enumerate(CACHED_KG)}

            def adj_tile_slice(n_i, kg, c0, c1):
                """AP for fp8 columns [c0:c1) of cached tile (n_i, kg)."""
                ci = cache_idx[kg]
                if ci < ncd:
                    off = (n_i * ncd + ci) * tw
                    return cache_sb[:, off + c0:off + c1]
                off = (n_i * (len(CACHED_KG) - ncd) + ci - ncd) * tw
                return xt_sb[:, off + c0:off + c1]

            # ---- pass A: 2^(sa+sx) h1T' = Delta.T @ adjT_fp8 + mt.T @ rsum
            # h1t[64s + h, a*iw + ii] = h1 for i-chunk (2a+s) (strip s in
            # array columns [64s, 64s+64), both strips share one psum bank)
            h1t_sb = h1t_pool.tile([nstrip * n_hid, npair * iw], F32)
            gl_sb = g_pool.tile([P, 2 * cfg.mcl], F32)
            gf_sb = [g_pool.tile([P, 2 * cfg.n_cores * hmc], F32,
                                 name=f"gf_sb{a}") for a in range(npair)]
            g_sb = [g_pool.tile([P, 2 * cfg.n_cores * hmc], FP8,
                                name=f"g_sb{a}") for a in range(npair)]
            for a in range(npair):
                # one psum bank per strip (partition 0): DoubleRow weights
                # occupy 2M=128 array columns, so the two strips cannot be
                # column-packed into one bank via tile_position.
                psA = [psA_pool.tile([n_hid, iw], F32, tag=f"psA{s}",
                                     name=f"psA{s}") for s in range(nstrip)]
                for idx, kg in enumerate(seqA):
                    rhss = []
                    for s in range(nstrip):
                        n_i = nstrip * a + s
                        if kg in cache_idx:
                            nc.sync.dma_start(
                                out=adj_tile_slice(n_i, kg, 0, tw),
                                in_=adjt_h[n_i, kg])
                            rhss.append(
                                lambda c0, c1, n_i=n_i, kg=kg:
                                adj_tile_slice(n_i, kg, c0, c1))
                        else:
                            at = adj_pool.tile([P, tw], FP8, tag="at")
                            nc.sync.dma_start(out=at[:, :], in_=adjt_h[n_i, kg])
                            rhss.append(
                                lambda c0, c1, at=at: at[:, c0:c1])
                    for kl in range(0, kpg, 2):
                        # DoubleRow: chunks (kg,kl) and (kg,kl+1) in one
                        # instruction — consecutive kl = consecutive node
                        # chunks, so both Delta and the adj tile are already
                        # plane-major-contiguous.
                        k = cfg.nchunk(kg, kl)
                        for s in range(nstrip):
                            nc.tensor.matmul(
                                psA[s][:, :],
                                lhsT=xw1_sb[:, k * n_hid:(k + 2) * n_hid]
                                .rearrange("p (two f) -> p two f", two=2),
                                rhs=rhss[s](kl * iw, (kl + 2) * iw)
                                .rearrange("p (two f) -> p two f", two=2),
                                start=(idx == 0 and kl == 0), stop=False,
                                perf_mode=DR,
                            )
                for s in range(nstrip):
                    nc.tensor.matmul(
                        psA[s][:, :],
                        lhsT=mt_sb[:, :],
                        rhs=rs_sb[:, (nstrip * a + s) * iw:(nstrip * a + s + 1) * iw],
                        start=False, stop=True,
                    )
                    # h1 = relu(2^-(sa+sx) * psA + b1), exact descale in fp32
                    nc.scalar.activation(
                        h1t_sb[s * n_hid:(s + 1) * n_hid,
                               a * iw:(a + 1) * iw], psA[s][:, :],
                        mybir.ActivationFunctionType.Relu,
                        bias=b1_sb[:n_hid, :],
                        scale=float(2.0 ** -(cfg.sa + cfg.sx)),
                    )
                # ---- stage 3 for this pair: delta_g = h1 @ W2 - c (fp32)
                for s in range(nstrip):
                    for ml in range(iw // P):
                        m = (nstrip * a + s) * (iw // P) + ml
                        ps3 = ps3_pool.tile([P, 2], F32, tag="ps3")
                        nc.tensor.matmul(
                            ps3[:, :],
                            lhsT=h1t_sb[s * n_hid:(s + 1) * n_hid,
                                        a * iw + ml * P:a * iw + (ml + 1) * P],
                            rhs=w2_sb[s * n_hid:(s + 1) * n_hid, :],
                            start=True, stop=True,
                        )
                        nc.vector.tensor_sub(
                            gl_sb[:, 2 * m:2 * m + 2], ps3[:, :], c2_sb[:, :])
                # ---- AllGather this pair's delta_g.  g_in rides the SWDGE
                # (Pool-engine) path: the HWDGE lanes serialize round-robin
                # with the adj stream, which would delay this tiny copy ~12us.
                nc.gpsimd.dma_start(
                    out=g_in[a][:, :],
                    in_=gl_sb[:, 2 * a * hmc:2 * (a + 1) * hmc])
                nc.gpsimd.collective_compute(
                    "AllGather", mybir.AluOpType.bypass,
                    ins=[g_in[a][:, :]], outs=[g_out[a][:, :]],
                    replica_groups=[list(range(cfg.n_cores))],
                )
                # g_out[a][(r*128+p), 2*m+t] -> gf[a][p, (r*hmc+m)*2+t]
                # SWDGE again: on a HWDGE lane this copy's Collectives wait
                # would head-of-line-block the pass-B adj prefetch behind it.
                nc.gpsimd.dma_start(
                    out=gf_sb[a][:, :].rearrange(
                        "p (r c) -> p r c", r=cfg.n_cores),
                    in_=g_out[a][:, :].rearrange("(r p) c -> p r c", p=P))

            # fp8 converts AFTER the pair loop: the scalar queue is in-order,
            # and convert-0 (gated on AG0) emitted before relu-1 would block
            # relu-1 -> stage-3 -> AG1 by several us.
            for a in range(npair):
                nc.scalar.activation(
                    g_sb[a][:, :], gf_sb[a][:, :],
                    mybir.ActivationFunctionType.Copy,
                    scale=float(2 ** cfg.sd))

            # ---- pass B: all ni i-chunks packed into ONE [128, iw] psum bank
            # via PE column-tiling: strip j (array cols [32j, 32j+32)) computes
            # i-chunk j.  2^(sa+sd) h2T'[t, i] lands at psum[32j + t, ii].
            # lhsT for chunk (kg, kl): g_sb[h][:, 2*(cc*kpg+kl) : +2].
            psB = psB_pool.tile([P, iw], F32)
            for idx, kg in enumerate(seqB):
                cc, h = kg % cfg.n_cores, kg // cfg.n_cores
                rhss = []
                for n_i in range(cfg.ni):
                    if kg in cache_idx:                  # cached from pass A
                        rhss.append(
                            lambda c0, c1, n_i=n_i, kg=kg:
                            adj_tile_slice(n_i, kg, c0, c1))
                    else:
                        at = adj_pool.tile([P, tw], FP8, tag="at")
                        nc.sync.dma_start(out=at[:, :], in_=adjt_h[n_i, kg])
                        rhss.append(lambda c0, c1, at=at: at[:, c0:c1])
                for kl in range(kpg):
                    gcol = 2 * (cc * kpg + kl)
                    for n_i in range(cfg.ni):
                        nc.tensor.matmul(
                            psB[32 * n_i:32 * n_i + 2, :],
                            lhsT=g_sb[h][:, gcol:gcol + 2],
                            rhs=rhss[n_i](kl * iw, (kl + 1) * iw),
                            start=(idx == 0 and kl == 0), stop=False,
                            tile_position=(0, 32 * n_i),
                            skip_group_check=True,
                        )
            for n_i in range(cfg.ni):
                nc.tensor.matmul(
                    psB[32 * n_i:32 * n_i + 2, :],
                    lhsT=ct_sb[:, :],
                    rhs=rs_sb[:, n_i * iw:(n_i + 1) * iw],
                    start=False, stop=True,
                    tile_position=(0, 32 * n_i),
                    skip_group_check=True,
                )
            # per-partition max over the free axis in ONE reduce (partitions
            # are independent; the host only reads rows 32j + t, the rest is
            # harmless junk from unwritten psum partitions)
            mxsb = mx_pool.tile([P, 1], F32)
            nc.vector.reduce_max(
                mxsb[:, :], psB[:, :], axis=mybir.AxisListType.X)
            mxo = mx_pool.tile([P, 1], F32)
            nc.scalar.mul(mxo[:, :], mxsb[:, :], float(2.0 ** -(cfg.sa + cfg.sd)))
            nc.sync.dma_start(out=out_h[:, :], in_=mxo[:, :])
    nc.compile()
    return nc


def shard_inputs(cfg: Cfg, x, adj, W1, b1, W2):
    """Host-side prep: pre-tile + quantize, and build the exactness sidecars
    (see module docstring)."""
    x = np.asarray(x, dtype=np.float32)
    adj = np.asarray(adj, dtype=np.float32)

    sxf = np.float32(2.0 ** cfg.sx)
    # xt[mg, p, ml, k, c] = fp8(2^sx * x)[128*(mg*mpg+ml)+c, 128*k+p]
    xb = (x * sxf).astype(FP8_NP)
    assert np.isfinite(xb.astype(np.float32)).all()
    xt = xb.reshape(cfg.nmg, cfg.mpg, P, cfg.fkc, P).transpose(0, 4, 1, 3, 2)
    xt = np.ascontiguousarray(xt).reshape(cfg.nmg, P, cfg.mpg * cfg.fkc * P)

    W1f = np.asarray(W1, dtype=np.float32)
    b1f = np.asarray(b1, dtype=np.float32)
    W2f = np.asarray(W2, dtype=np.float32)
    w1b = W1f.astype(FP8_NP)
    w1 = np.ascontiguousarray(w1b.reshape(cfg.fkc, P, cfg.n_hid))
    # b1/W2 duplicated into both partition halves for the pass-A 2x packing
    b1d = np.ascontiguousarray(
        np.concatenate([b1f, b1f]).reshape(2 * cfg.n_hid, 1))
    w2 = np.ascontiguousarray(np.vstack([W2f, W2f]))

    # --- pass-A sidecars: exact simulation of the device quantizations.
    # device stage-1 product (2^sx-scaled), bf16 operands, f32 accumulate:
    xW1_dev = xb.astype(np.float32) @ w1b.astype(np.float32)     # 2^sx-scaled
    m_dev = xW1_dev.mean(axis=0, dtype=np.float64).astype(np.float32)
    Q = xW1_dev - m_dev                                          # device Delta
    Qq = Q.astype(FP8_NP).astype(np.float32)                     # fp8(Delta)
    assert np.isfinite(Qq).all(), "Delta overflows fp8 range"
    eps = (Qq - Q).mean(axis=0, dtype=np.float64).astype(np.float32)
    m_true = (x.mean(axis=0, dtype=np.float64).astype(np.float32) @ W1f)
    # correction lhsT: in 2^(sa+sx)-scaled psum units per unit rowsum
    mt_val = (m_true * sxf - eps) * np.float32(2.0 ** cfg.sa)
    m2 = np.ascontiguousarray(np.broadcast_to(
        np.tile(m_dev, 2), (P, 2 * cfg.n_hid)).astype(np.float32))
    mt = np.ascontiguousarray(mt_val.reshape(1, cfg.n_hid).astype(np.float32))

    # --- pass-B center estimate from a row subsample (any c is exact;
    # closer c => smaller |delta_g| => less fp8 noise)
    idx = np.arange(0, cfg.n, max(1, cfg.n // 256))
    g_sub = np.maximum(adj[idx] @ (xW1_dev / sxf) + b1f, 0.0) @ W2f
    c_est = g_sub.mean(axis=0).astype(np.float32)                # [2]
    c2 = np.ascontiguousarray(np.broadcast_to(c_est, (P, 2)).astype(np.float32))
    ct = np.ascontiguousarray(
        (c_est * np.float32(2.0 ** (cfg.sa + cfg.sd))).reshape(1, 2))
    rsum = adj.sum(axis=1, dtype=np.float64).astype(np.float32)  # [n]

    saf = np.float32(2.0 ** cfg.sa)
    in_maps = []
    for c in range(cfg.n_cores):
        shard = adj[c * cfg.rows:(c + 1) * cfg.rows, :]
        # a[n_i, kg=(h, cc), p, kl, ii] = shard[iw*n_i+ii, node(kg,kl,p)]
        # node(kg,kl,p) = cc*2048 + h*1024 + kl*128 + p
        a6 = shard.reshape(cfg.ni, cfg.iw,
                           cfg.n_cores, 2, cfg.kpg, P)   # [ni,ii,cc,h,kl,p]
        a6 = a6.transpose(0, 3, 2, 5, 4, 1)              # [ni,h,cc,p,kl,ii]
        a2 = np.ascontiguousarray((a6 * saf).astype(FP8_NP)).reshape(
            cfg.ni, cfg.nkg, P, cfg.kpg * cfg.iw)
        rs = np.ascontiguousarray(
            rsum[c * cfg.rows:(c + 1) * cfg.rows].reshape(1, cfg.rows))
        in_maps.append({"adjt3": a2, "xt": xt, "w1": w1, "b1": b1d,
                        "w2": w2, "c2": c2, "ct": ct, "m2": m2, "mt": mt,
                        "rsum": rs})
    return in_maps


def finish_on_host(cfg: Cfg, per_core_out, b2, W3, b3):
    """per_core_out: [n_cores, 128] device outputs (strip j's maxima at
    [32j + t]) -> [1,1,1] final output."""
    b2 = np.asarray(b2, dtype=np.float32)
    W3 = np.asarray(W3, dtype=np.float32)
    b3 = np.asarray(b3, dtype=np.float32)
    strips = np.stack([per_core_out[:, 32 * j:32 * j + 2]
                       for j in range(cfg.ni)])          # [ni, n_cores, 2]
    pooled = strips.max(axis=(0, 1)).astype(np.float32) + b2       # [2]
    out = pooled[None, None, :] @ W3.T + b3                        # [1,1,1]
    return out.astype(np.float32)


_NC_CACHE: dict = {}
LAST_RESULT = None  # BassKernelResults of the most recent run (for test.py)


def kernel(x, adj, W1, b1, W2, b2, W3, b3):
    cfg = Cfg()
    x = np.asarray(x)
    assert x.shape == (cfg.n, cfg.n_feat), x.shape
    if "nc" not in _NC_CACHE:
        _NC_CACHE["nc"] = build_nc(cfg)
    nc = _NC_CACHE["nc"]

    in_maps = shard_inputs(cfg, x, adj, W1, b1, W2)
    trace = os.environ.get("GCN_TRACE", "0") == "1"
    res = run_bass_kernel_spmd(
        nc, in_maps, core_ids=list(range(cfg.n_cores)), trace=trace)
    global LAST_RESULT
    LAST_RESULT = res
    per_core = np.stack(
        [np.asarray(r["out"][:, 0], dtype=np.float32) for r in res.results])
    return finish_on_host(cfg, per_core, b2, W3, b3)


# revision 70
# speedup vs baseline: 1.1452x; 1.0300x over previous
"""GCN forward (2x graph-conv + global max-pool + linear) on 8 TRN2 NeuronCores.

Reference computation (N=16384 nodes, 256 feats, 64 hid):
    h1 = relu(adj @ (x @ W1) + b1)          [N, 64]
    h2 = adj @ (h1 @ W2) + b2               [N, 2]
    out = max(h2, axis=0) @ W3.T + b3       [1, 1, 1]

Distribution: row-shard adj over the 8 cores (core c owns output rows
[c*2048, (c+1)*2048)).  Each core:
  stage 1: Delta = bf16(16x)@bf16(W1) - m2, stored fp8  (replicated, tiny)
  pass A : h1T' = Delta.T @ adjT_fp8 + mt.T@rsum        [64, 2048] scaled
           bias/relu fused on psum evacuation (exact descale via act scale)
  stage 3: g_local = h1 @ W2 (fp32); delta_g = g_local - c
  AllGather delta_g (split per strip-pair) -> on-device collective
  pass B : h2T' = delta_g.T @ adjT_fp8 + ct.T@rsum      4x column-packed in
           one PSUM bank via tile_position; per-i-chunk max -> [128, 1]
Host: unpack/max over strips and cores, + b2, @ W3.T + b3.

Perf structure (~224-238us vs the 305us baseline; run-to-run spread is
cross-core collective skew; see comments inline):
  - adj streams as fp8e4m3 in 512 KiB tiles; k-chunks are grouped
    kg = h*8 + cc (cc = owning core, h = half) so that strip-pair `a`
    of every core produces exactly the h=a delta_g chunks -> the
    AllGather is split per pair; AG0 completes under pass-A pair 1.
  - x and W1 also stream as fp8 (the sidecars cancel the quantization
    systematics exactly); stage 1 and pass A use DoubleRow matmuls
    (fp8 fast path, 256-deep contraction per instruction).
  - pass A consumes kg groups so its first matmul data-depends on
    stage 1's LAST output chunk: stage 1 (fp8 M=128) and pass A never
    interleave on the PE (interleaving different matmul flavors drops
    the fp8 double-pump rate ~3x).  Each pair ends on CACHED groups so
    the DMA ring pre-streams pass B's first tiles during the tail.
  - the whole h=1 adj half (32 tiles = 16 MiB) stays resident in SBUF
    between the passes (24 tiles dedicated + 8 aliasing the dead xt
    buffer), cutting the 64 MiB/core adj stream to ~52 MiB.  Pass B
    opens on STREAMED h0 groups (AG0 long done) so the DMA engines
    never idle while AG1 completes; the AG1-gated h1 half runs purely
    from cache at the end.
  - HW-DGE copies are serialized round-robin onto 8 semaphore lanes;
    every latency-critical small copy (g / gf bounces) rides the SWDGE
    (Pool-engine) path instead so it is never queued behind the adj
    stream, and consts are issued before xt before adj so each lane
    serves them in dependency order.

fp8 noise is harmless because both passes compute the large mean
component exactly in fp32 via host-side sidecars:
  - rsum: exact f32 row-sums of adj (the only O(N^2) host work),
  - mt/ct: column-means, with exact cancellation of every quantization
    systematic (host simulates the device's bf16/fp8 quantization
    exactly); only sqrt(N)-damped random noise survives.
"""

import os
import sys

sys.path.insert(0, "/opt/trn_rl_repo")

import numpy as np
import ml_dtypes


def _install_ntff_hook_shim():
    """The image's `antenv` lacks `axon_hooks`, which bass_utils imports for
    trace=True under axon. Provide it, wired to the PJRT .so's NRT-profile
    C ABI (same thing trn_boot would have registered)."""
    import types
    if "antenv.axon_hooks" in sys.modules:
        return
    try:
        import antenv  # noqa: F401
        from trn_agent_boot.trn_boot import _ntff_profile_via_ctypes
        mod = types.ModuleType("antenv.axon_hooks")
        _state = {"hook": _ntff_profile_via_ctypes("/opt/axon/libaxon_pjrt.so")}
        mod.set_axon_ntff_profile_hook = lambda h: _state.update(hook=h)
        mod.get_axon_ntff_profile_hook = lambda: _state["hook"]
        sys.modules["antenv.axon_hooks"] = mod
    except Exception:
        pass


_install_ntff_hook_shim()

import concourse.bass as bass
import concourse.mybir as mybir
import concourse.tile as tile
from concourse import bacc
from concourse.bass_utils import run_bass_kernel_spmd

BF16_NP = ml_dtypes.bfloat16
FP8_NP = ml_dtypes.float8_e4m3

P = 128          # partition dim
N_CORES = 8
N_NODES = 16384
N_FEAT = 256
N_HID = 64


class Cfg:
    def __init__(self, n=N_NODES, n_feat=N_FEAT, n_hid=N_HID, n_cores=N_CORES,
                 iw=512, kpg=8, mpg=8, adj_bufs=12, n_cache_kg=4,
                 sa=21, sd=10, sx=4):
        self.n, self.n_feat, self.n_hid, self.n_cores = n, n_feat, n_hid, n_cores
        self.rows = n // n_cores       # output rows per core
        self.iw = iw                   # i-tile width (psum free dim)
        self.kpg = kpg                 # k-chunks (128 nodes each) per adj tile
        self.mpg = mpg                 # m-chunks per xt DMA slice
        self.kc = n // P               # contraction chunks (over all nodes)
        self.nkg = self.kc // kpg      # adj tile groups (= 2 halves x 8 cores)
        self.ni = self.rows // iw      # i-chunks per core
        self.mc = n // P               # stage-1 m-chunks (all nodes)
        self.nmg = self.mc // mpg      # xt DMA slices
        self.mcl = self.rows // P      # stage-3 m-chunks (local rows)
        self.fkc = n_feat // P         # feature contraction chunks
        self.adj_bufs = adj_bufs       # streaming ring depth (512 KiB tiles)
        self.n_cache_kg = n_cache_kg   # kg groups cached in SBUF for pass B
        # fp8 scales (powers of 2, exact): adj x2^sa keeps max < 240;
        # stage-1 operands x2^sx so Delta_fp8 is 2^sx-scaled; pass-B delta
        # x2^sd on device.  psA holds 2^(sa+sx)*h1T', psB 2^(sa+sd)*h2T'.
        self.sa = sa
        self.sd = sd
        self.sx = sx
        assert self.rows % iw == 0 and self.kc % kpg == 0 and self.mc % mpg == 0
        assert self.nkg == 2 * n_cores      # kg = h*8 + cc layout
        assert self.iw % P == 0 and self.ni in (1, 2, 4)
        assert self.n_cache_kg <= n_cores

    # node-chunk index (into Delta / delta_g, natural node order) covered by
    # (kg, kl): kg = h*n_cores + cc covers nodes cc*2048 + h*1024 + kl*128 + p
    def nchunk(self, kg, kl):
        cc, h = kg % self.n_cores, kg // self.n_cores
        return cc * (self.rows // P) + h * self.kpg + kl


# kg groups whose tiles stay resident in SBUF between the passes (32 tiles =
# 16 MiB): most of h=0, plus the last two h=1 groups.  Pass B orders its
# consumption so that while it waits for AG1 it is (a) computing from cache
# and (b) still has streamed-but-AG0-ready groups keeping the DMA busy.
# The whole h=1 half (kg 8..15, x4 strips = 32 tiles = 16 MiB) stays resident
# in SBUF between the passes.  Pass B then opens on STREAMED h0 groups —
# whose delta_g (AG0) completed while pass A was still running — so the DMA
# engines never idle waiting for AG1; the AG1-gated h1 half runs purely from
# cache at the end.  The first CACHE_DEDICATED entries live in the dedicated
# cache buffer; the last two (kg 9, 8 — the cached tiles pass A consumes
# LAST) alias the xt buffer, whose WAR-gated DMAs must not
# head-of-line-block the hw-DMA lanes early on.
CACHED_KG = (8, 9, 10, 11, 12, 13, 14, 15)
CACHE_DEDICATED = 6


def _passA_kg_order(cfg: Cfg):
    """ASCENDING node-chunk order: pass A's matmul for chunk k data-depends
    on stage 1's chunk k, so the scheduler interleaves pass A INTO stage 1
    as Delta chunks appear — pass A starts ~20us earlier than a full
    serialization.  (Both are fp8 DoubleRow now, so the PE-mode-thrash
    penalty that once forced serialization no longer applies; and even a
    degraded matmul rate outpaces the DMA stream.)  The xt-ALIASED cache
    groups (kg 14, 15) come last: their DMAs WAR-wait on stage 1's xt
    reads, and issued early they would head-of-line-block the hw-DMA
    lanes."""
    return [0, 8, 1, 9, 2, 10, 3, 11, 4, 12, 5, 13, 6, 14, 7, 15]


def _passB_kg_order(cfg: Cfg):
    """Streamed h0 (AG0 already done) first; cached h1 (AG1-gated) groups
    interleave from the point AG1 is typically complete (~6 streamed groups
    in), filling PE gaps in the DMA-paced stretch.  If AG1 is late the PE
    stalls at kg 8 but the DMA ring keeps streaming, so the worst case ties
    the non-interleaved order."""
    return [0, 1, 2, 3, 4, 5, 8, 6, 9, 10, 7, 11, 12, 13, 14, 15]


def build_nc(cfg: Cfg) -> bass.Bass:
    BF = mybir.dt.bfloat16
    F32 = mybir.dt.float32
    FP8 = mybir.dt.float8e4
    n_hid, iw, kpg, fkc = cfg.n_hid, cfg.iw, cfg.kpg, cfg.fkc
    tw = kpg * iw                       # adj tile free width (4096)

    nc = bacc.Bacc("TRN2", target_bir_lowering=False)
    # adjt[n_i, kg][p, kl*iw + ii] = 2^sa * adjT_shard[node(kg,kl,p),
    # iw*n_i+ii] in fp8e4m3 (node(kg,kl,p) per Cfg.nchunk).
    adjt_h = nc.declare_dram_parameter(
        "adjt3", [cfg.ni, cfg.nkg, P, tw], FP8, isOutput=False)
    # xt[mg][p, (ml*fkc+k)*128 + c] = fp8(2^sx * x)[128*(mg*mpg+ml)+c, 128*k+p]
    # (stage 1 is replicated: exchanging Delta via collective_compute costs
    # 30-60us for a 1 MiB gather — worse than streaming the full 4 MiB x.
    # x and W1 stream as fp8: their quantization systematics are cancelled
    # exactly by the host-side sidecars, the random part is CLT-damped.)
    xt_h = nc.declare_dram_parameter(
        "xt", [cfg.nmg, P, cfg.mpg * fkc * P], FP8, isOutput=False)
    w1_h = nc.declare_dram_parameter("w1", [fkc, P, n_hid], FP8, isOutput=False)
    b1_h = nc.declare_dram_parameter("b1", [2 * n_hid, 1], F32, isOutput=False)
    w2_h = nc.declare_dram_parameter("w2", [2 * n_hid, 2], F32, isOutput=False)
    # host-side exactness sidecars (see module docstring):
    #   m2  = col-means of the device product bf16(2^sx x)@bf16(W1)  [scaled]
    #   mt  = (true col-means of x@W1 minus fp8(Delta) quantization bias)
    #         * 2^(sa+sx)   -- pass-A correction lhsT
    #   c2/ct = pass-B center estimate (c2 plain, ct * 2^(sa+sd))
    #   rsum  = exact f32 row-sums of this core's adj rows
    c2_h = nc.declare_dram_parameter("c2", [P, 2], F32, isOutput=False)
    ct_h = nc.declare_dram_parameter("ct", [1, 2], F32, isOutput=False)
    # m2 duplicated into both halves for the paired-bank stage-1 evacuation
    m2_h = nc.declare_dram_parameter("m2", [P, 2 * n_hid], F32, isOutput=False)
    mt_h = nc.declare_dram_parameter("mt", [1, n_hid], F32, isOutput=False)
    rs_h = nc.declare_dram_parameter("rsum", [1, cfg.rows], F32, isOutput=False)
    # out[32j + t] = max over i-chunk j (valid for j < ni, t < 2)
    out_h = nc.declare_dram_parameter("out", [P, 1], F32, isOutput=True)

    # collective bounce buffers, one pair per strip-pair:
    # g_in[a][p, 2*m+t] = delta_g_local[a*1024 + 128*m + p, t],  m in [0,8)
    npair = max(1, cfg.ni // 2)
    nstrip = min(2, cfg.ni)
    hmc = cfg.mcl // npair              # local m-chunks per pair (8)
    g_in = [nc.dram_tensor(f"g_in{a}", [P, 2 * hmc], F32)
            for a in range(npair)]
    g_out = [nc.dram_tensor(f"g_out{a}", [P * cfg.n_cores, 2 * hmc], F32,
                            addr_space="Shared") for a in range(npair)]

    seqA = _passA_kg_order(cfg)
    seqB = _passB_kg_order(cfg)

    with tile.TileContext(nc, num_cores=cfg.n_cores) as tc:
        with (
            tc.tile_pool(name="const", bufs=1) as const_pool,
            tc.tile_pool(name="xtp", bufs=1) as xt_pool,
            tc.tile_pool(name="xw1p", bufs=1) as xw1_pool,
            tc.tile_pool(name="h1tp", bufs=1) as h1t_pool,
            tc.tile_pool(name="cachep", bufs=1) as cache_pool,
            tc.tile_pool(name="adjp", bufs=cfg.adj_bufs) as adj_pool,
            tc.tile_pool(name="gp", bufs=1) as g_pool,
            tc.tile_pool(name="mxp", bufs=1) as mx_pool,
            tc.tile_pool(name="ps1p", bufs=3, space="PSUM") as ps1_pool,
            tc.tile_pool(name="psAp", bufs=1, space="PSUM") as psA_pool,
            tc.tile_pool(name="ps3p", bufs=2, space="PSUM") as ps3_pool,
            tc.tile_pool(name="psBp", bufs=1, space="PSUM") as psB_pool,
        ):
            # ---- constants first: every stage-1 matmul needs w1, so it must
            # land on a lane AHEAD of the xt flood (lanes execute in order).
            w1_sb = const_pool.tile([P, fkc * n_hid], FP8)
            for k in range(fkc):
                nc.sync.dma_start(
                    out=w1_sb[:, k * n_hid:(k + 1) * n_hid], in_=w1_h[k])
            b1_sb = const_pool.tile([2 * n_hid, 1], F32)
            nc.sync.dma_start(out=b1_sb[:, :], in_=b1_h[:, :])
            w2_sb = const_pool.tile([2 * n_hid, 2], F32)
            nc.sync.dma_start(out=w2_sb[:, :], in_=w2_h[:, :])
            c2_sb = const_pool.tile([P, 2], F32)
            nc.sync.dma_start(out=c2_sb[:, :], in_=c2_h[:, :])
            ct_sb = const_pool.tile([1, 2], F32)
            nc.sync.dma_start(out=ct_sb[:, :], in_=ct_h[:, :])
            m2_sb = const_pool.tile([P, 2 * n_hid], F32)
            nc.sync.dma_start(out=m2_sb[:, :], in_=m2_h[:, :])
            mt_sb = const_pool.tile([1, n_hid], F32)
            nc.sync.dma_start(out=mt_sb[:, :], in_=mt_h[:, :])
            rs_sb = const_pool.tile([1, cfg.rows], F32)
            nc.sync.dma_start(out=rs_sb[:, :], in_=rs_h[:, :])

            # ---- xt next: 2 partition-split copies per slice (32 total) so
            # slice 0 lands on two hw-DMA lanes in parallel and stage 1
            # starts long before the full 4 MiB is in.  All 32 copies sit
            # ahead of the adj flood in the lane rotation.
            xt_sb = xt_pool.tile([P, cfg.nmg * cfg.mpg * fkc * P], FP8)
            xg = cfg.mpg * fkc * P
            for mg in range(cfg.nmg):
                for ph in range(2):
                    nc.sync.dma_start(
                        out=xt_sb[64 * ph:64 * (ph + 1),
                                  mg * xg:(mg + 1) * xg],
                        in_=xt_h[mg][64 * ph:64 * (ph + 1), :])

            # ---- stage 1: Delta = (2^sx x)@W1 - m2, stored fp8 node-major.
            # DoubleRow contracts both 128-feature chunks in one instruction
            # (fp8 fast path); two m-chunks share one psum bank so a single
            # double-width DVE subtract evacuates both (the sub chain would
            # otherwise pace stage 1).
            DR = mybir.MatmulPerfMode.DoubleRow
            xw1_sb = xw1_pool.tile([P, cfg.mc * n_hid], FP8)
            for mp in range(cfg.mc // 2):
                ps1 = ps1_pool.tile([P, 2 * n_hid], F32, tag="ps1")
                for j in range(2):
                    m = 2 * mp + j
                    nc.tensor.matmul(
                        ps1[:, j * n_hid:(j + 1) * n_hid],
                        lhsT=xt_sb[:, m * fkc * P:(m + 1) * fkc * P].rearrange(
                            "p (two f) -> p two f", two=2),
                        rhs=w1_sb[:, :].rearrange("p (two f) -> p two f", two=2),
                        start=True, stop=True, perf_mode=DR,
                    )
                nc.vector.tensor_sub(
                    xw1_sb[:, 2 * mp * n_hid:2 * (mp + 1) * n_hid], ps1[:, :],
                    m2_sb[:, :])

            # ---- SBUF cache for adj tiles reused by pass B (CACHED_KG x 4
            # strips = 32 tiles = 16 MiB): 24 in a dedicated buffer, 8
            # aliasing the xt buffer (dead after stage 1; the framework
            # WAR-orders each aliased cache DMA after stage 1's last read of
            # the overlapping columns).
            ncd = CACHE_DEDICATED
            cache_sb = cache_pool.tile([P, cfg.ni * ncd * tw], FP8)
            cache_idx = {kg: i for i, kg in enumerate(CACHED_KG)}

            def adj_tile_slice(n_i, kg, c0, c1):
                """AP for fp8 columns [c0:c1) of cached tile (n_i, kg)."""
                ci = cache_idx[kg]
                if ci < ncd:
                    off = (n_i * ncd + ci) * tw
                    return cache_sb[:, off + c0:off + c1]
                off = (n_i * (len(CACHED_KG) - ncd) + ci - ncd) * tw
                return xt_sb[:, off + c0:off + c1]

            # ---- pass A: 2^(sa+sx) h1T' = Delta.T @ adjT_fp8 + mt.T @ rsum
            # h1t[64s + h, a*iw + ii] = h1 for i-chunk (2a+s) (strip s in
            # array columns [64s, 64s+64), both strips share one psum bank)
            h1t_sb = h1t_pool.tile([nstrip * n_hid, npair * iw], F32)
            gl_sb = g_pool.tile([P, 2 * cfg.mcl], F32)
            gf_sb = [g_pool.tile([P, 2 * cfg.n_cores * hmc], F32,
                                 name=f"gf_sb{a}") for a in range(npair)]
            g_sb = [g_pool.tile([P, 2 * cfg.n_cores * hmc], FP8,
                                name=f"g_sb{a}") for a in range(npair)]
            for a in range(npair):
                # one psum bank per strip (partition 0): DoubleRow weights
                # occupy 2M=128 array columns, so the two strips cannot be
                # column-packed into one bank via tile_position.
                psA = [psA_pool.tile([n_hid, iw], F32, tag=f"psA{s}",
                                     name=f"psA{s}") for s in range(nstrip)]
                for idx, kg in enumerate(seqA):
                    rhss = []
                    for s in range(nstrip):
                        n_i = nstrip * a + s
                        if kg in cache_idx:
                            nc.sync.dma_start(
                                out=adj_tile_slice(n_i, kg, 0, tw),
                                in_=adjt_h[n_i, kg])
                            rhss.append(
                                lambda c0, c1, n_i=n_i, kg=kg:
                                adj_tile_slice(n_i, kg, c0, c1))
                        else:
                            at = adj_pool.tile([P, tw], FP8, tag="at")
                            nc.sync.dma_start(out=at[:, :], in_=adjt_h[n_i, kg])
                            rhss.append(
                                lambda c0, c1, at=at: at[:, c0:c1])
                    for kl in range(0, kpg, 2):
                        # DoubleRow: chunks (kg,kl) and (kg,kl+1) in one
                        # instruction — consecutive kl = consecutive node
                        # chunks, so both Delta and the adj tile are already
                        # plane-major-contiguous.
                        k = cfg.nchunk(kg, kl)
                        for s in range(nstrip):
                            nc.tensor.matmul(
                                psA[s][:, :],
                                lhsT=xw1_sb[:, k * n_hid:(k + 2) * n_hid]
                                .rearrange("p (two f) -> p two f", two=2),
                                rhs=rhss[s](kl * iw, (kl + 2) * iw)
                                .rearrange("p (two f) -> p two f", two=2),
                                start=(idx == 0 and kl == 0), stop=False,
                                perf_mode=DR,
                            )
                for s in range(nstrip):
                    nc.tensor.matmul(
                        psA[s][:, :],
                        lhsT=mt_sb[:, :],
                        rhs=rs_sb[:, (nstrip * a + s) * iw:(nstrip * a + s + 1) * iw],
                        start=False, stop=True,
                    )
                    # h1 = relu(2^-(sa+sx) * psA + b1), exact descale in fp32
                    nc.scalar.activation(
                        h1t_sb[s * n_hid:(s + 1) * n_hid,
                               a * iw:(a + 1) * iw], psA[s][:, :],
                        mybir.ActivationFunctionType.Relu,
                        bias=b1_sb[:n_hid, :],
                        scale=float(2.0 ** -(cfg.sa + cfg.sx)),
                    )
                # ---- stage 3 for this pair: delta_g = h1 @ W2 - c (fp32)
                for s in range(nstrip):
                    for ml in range(iw // P):
                        m = (nstrip * a + s) * (iw // P) + ml
                        ps3 = ps3_pool.tile([P, 2], F32, tag="ps3")
                        nc.tensor.matmul(
                            ps3[:, :],
                            lhsT=h1t_sb[s * n_hid:(s + 1) * n_hid,
                                        a * iw + ml * P:a * iw + (ml + 1) * P],
                            rhs=w2_sb[s * n_hid:(s + 1) * n_hid, :],
                            start=True, stop=True,
                        )
                        nc.vector.tensor_sub(
                            gl_sb[:, 2 * m:2 * m + 2], ps3[:, :], c2_sb[:, :])
                # ---- AllGather this pair's delta_g.  g_in rides the SWDGE
                # (Pool-engine) path: the HWDGE lanes serialize round-robin
                # with the adj stream, which would delay this tiny copy ~12us.
                nc.gpsimd.dma_start(
                    out=g_in[a][:, :],
                    in_=gl_sb[:, 2 * a * hmc:2 * (a + 1) * hmc])
                nc.gpsimd.collective_compute(
                    "AllGather", mybir.AluOpType.bypass,
                    ins=[g_in[a][:, :]], outs=[g_out[a][:, :]],
                    replica_groups=[list(range(cfg.n_cores))],
                )
                # g_out[a][(r*128+p), 2*m+t] -> gf[a][p, (r*hmc+m)*2+t]
                # SWDGE again: on a HWDGE lane this copy's Collectives wait
                # would head-of-line-block the pass-B adj prefetch behind it.
                nc.gpsimd.dma_start(
                    out=gf_sb[a][:, :].rearrange(
                        "p (r c) -> p r c", r=cfg.n_cores),
                    in_=g_out[a][:, :].rearrange("(r p) c -> p r c", p=P))

            # fp8 converts AFTER the pair loop: the scalar queue is in-order,
            # and convert-0 (gated on AG0) emitted before relu-1 would block
            # relu-1 -> stage-3 -> AG1 by several us.
            for a in range(npair):
                nc.scalar.activation(
                    g_sb[a][:, :], gf_sb[a][:, :],
                    mybir.ActivationFunctionType.Copy,
                    scale=float(2 ** cfg.sd))

            # ---- pass B: all ni i-chunks packed into ONE [128, iw] psum bank
            # via PE column-tiling: strip j (array cols [32j, 32j+32)) computes
            # i-chunk j.  2^(sa+sd) h2T'[t, i] lands at psum[32j + t, ii].
            # lhsT for chunk (kg, kl): g_sb[h][:, 2*(cc*kpg+kl) : +2].
            psB = psB_pool.tile([P, iw], F32)
            for idx, kg in enumerate(seqB):
                cc, h = kg % cfg.n_cores, kg // cfg.n_cores
                rhss = []
                for n_i in range(cfg.ni):
                    if kg in cache_idx:                  # cached from pass A
                        rhss.append(
                            lambda c0, c1, n_i=n_i, kg=kg:
                            adj_tile_slice(n_i, kg, c0, c1))
                    else:
                        at = adj_pool.tile([P, tw], FP8, tag="at")
                        nc.sync.dma_start(out=at[:, :], in_=adjt_h[n_i, kg])
                        rhss.append(lambda c0, c1, at=at: at[:, c0:c1])
                for kl in range(kpg):
                    gcol = 2 * (cc * kpg + kl)
                    for n_i in range(cfg.ni):
                        nc.tensor.matmul(
                            psB[32 * n_i:32 * n_i + 2, :],
                            lhsT=g_sb[h][:, gcol:gcol + 2],
                            rhs=rhss[n_i](kl * iw, (kl + 1) * iw),
                            start=(idx == 0 and kl == 0), stop=False,
                            tile_position=(0, 32 * n_i),
                            skip_group_check=True,
                        )
            for n_i in range(cfg.ni):
                nc.tensor.matmul(
                    psB[32 * n_i:32 * n_i + 2, :],
                    lhsT=ct_sb[:, :],
                    rhs=rs_sb[:, n_i * iw:(n_i + 1) * iw],
                    start=False, stop=True,
                    tile_position=(0, 32 * n_i),
                    skip_group_check=True,
                )
            # per-partition max over the free axis in ONE reduce (partitions
            # are independent; the host only reads rows 32j + t, the rest is
            # harmless junk from unwritten psum partitions)
            mxsb = mx_pool.tile([P, 1], F32)
            nc.vector.reduce_max(
                mxsb[:, :], psB[:, :], axis=mybir.AxisListType.X)
            mxo = mx_pool.tile([P, 1], F32)
            nc.scalar.mul(mxo[:, :], mxsb[:, :], float(2.0 ** -(cfg.sa + cfg.sd)))
            nc.sync.dma_start(out=out_h[:, :], in_=mxo[:, :])
    nc.compile()
    return nc


def shard_inputs(cfg: Cfg, x, adj, W1, b1, W2):
    """Host-side prep: pre-tile + quantize, and build the exactness sidecars
    (see module docstring)."""
    x = np.asarray(x, dtype=np.float32)
    adj = np.asarray(adj, dtype=np.float32)

    sxf = np.float32(2.0 ** cfg.sx)
    # xt[mg, p, ml, k, c] = fp8(2^sx * x)[128*(mg*mpg+ml)+c, 128*k+p]
    xb = (x * sxf).astype(FP8_NP)
    assert np.isfinite(xb.astype(np.float32)).all()
    xt = xb.reshape(cfg.nmg, cfg.mpg, P, cfg.fkc, P).transpose(0, 4, 1, 3, 2)
    xt = np.ascontiguousarray(xt).reshape(cfg.nmg, P, cfg.mpg * cfg.fkc * P)

    W1f = np.asarray(W1, dtype=np.float32)
    b1f = np.asarray(b1, dtype=np.float32)
    W2f = np.asarray(W2, dtype=np.float32)
    w1b = W1f.astype(FP8_NP)
    w1 = np.ascontiguousarray(w1b.reshape(cfg.fkc, P, cfg.n_hid))
    # b1/W2 duplicated into both partition halves for the pass-A 2x packing
    b1d = np.ascontiguousarray(
        np.concatenate([b1f, b1f]).reshape(2 * cfg.n_hid, 1))
    w2 = np.ascontiguousarray(np.vstack([W2f, W2f]))

    # --- pass-A sidecars: exact simulation of the device quantizations.
    # device stage-1 product (2^sx-scaled), bf16 operands, f32 accumulate:
    xW1_dev = xb.astype(np.float32) @ w1b.astype(np.float32)     # 2^sx-scaled
    m_dev = xW1_dev.mean(axis=0, dtype=np.float64).astype(np.float32)
    Q = xW1_dev - m_dev                                          # device Delta
    Qq = Q.astype(FP8_NP).astype(np.float32)                     # fp8(Delta)
    assert np.isfinite(Qq).all(), "Delta overflows fp8 range"
    eps = (Qq - Q).mean(axis=0, dtype=np.float64).astype(np.float32)
    m_true = (x.mean(axis=0, dtype=np.float64).astype(np.float32) @ W1f)
    # correction lhsT: in 2^(sa+sx)-scaled psum units per unit rowsum
    mt_val = (m_true * sxf - eps) * np.float32(2.0 ** cfg.sa)
    m2 = np.ascontiguousarray(np.broadcast_to(
        np.tile(m_dev, 2), (P, 2 * cfg.n_hid)).astype(np.float32))
    mt = np.ascontiguousarray(mt_val.reshape(1, cfg.n_hid).astype(np.float32))

    # --- pass-B center estimate from a row subsample (any c is exact;
    # closer c => smaller |delta_g| => less fp8 noise)
    idx = np.arange(0, cfg.n, max(1, cfg.n // 256))
    g_sub = np.maximum(adj[idx] @ (xW1_dev / sxf) + b1f, 0.0) @ W2f
    c_est = g_sub.mean(axis=0).astype(np.float32)                # [2]
    c2 = np.ascontiguousarray(np.broadcast_to(c_est, (P, 2)).astype(np.float32))
    ct = np.ascontiguousarray(
        (c_est * np.float32(2.0 ** (cfg.sa + cfg.sd))).reshape(1, 2))
    rsum = adj.sum(axis=1, dtype=np.float64).astype(np.float32)  # [n]

    saf = np.float32(2.0 ** cfg.sa)
    in_maps = []
    for c in range(cfg.n_cores):
        shard = adj[c * cfg.rows:(c + 1) * cfg.rows, :]
        # a[n_i, kg=(h, cc), p, kl, ii] = shard[iw*n_i+ii, node(kg,kl,p)]
        # node(kg,kl,p) = cc*2048 + h*1024 + kl*128 + p
        a6 = shard.reshape(cfg.ni, cfg.iw,
                           cfg.n_cores, 2, cfg.kpg, P)   # [ni,ii,cc,h,kl,p]
        a6 = a6.transpose(0, 3, 2, 5, 4, 1)              # [ni,h,cc,p,kl,ii]
        a2 = np.ascontiguousarray((a6 * saf).astype(FP8_NP)).reshape(
            cfg.ni, cfg.nkg, P, cfg.kpg * cfg.iw)
        rs = np.ascontiguousarray(
            rsum[c * cfg.rows:(c + 1) * cfg.rows].reshape(1, cfg.rows))
        in_maps.append({"adjt3": a2, "xt": xt, "w1": w1, "b1": b1d,
                        "w2": w2, "c2": c2, "ct": ct, "m2": m2, "mt": mt,
                        "rsum": rs})
    return in_maps


def finish_on_host(cfg: Cfg, per_core_out, b2, W3, b3):
    """per_core_out: [n_cores, 128] device outputs (strip j's maxima at
    [32j + t]) -> [1,1,1] final output."""
    b2 = np.asarray(b2, dtype=np.float32)
    W3 = np.asarray(W3, dtype=np.float32)
    b3 = np.asarray(b3, dtype=np.float32)
    strips = np.stack([per_core_out[:, 32 * j:32 * j + 2]
                       for j in range(cfg.ni)])          # [ni, n_cores, 2]
    pooled = strips.max(axis=(0, 1)).astype(np.float32) + b2       # [2]
    out = pooled[None, None, :] @ W3.T + b3                        # [1,1,1]
    return out.astype(np.float32)


_NC_CACHE: dict = {}
LAST_RESULT = None  # BassKernelResults of the most recent run (for test.py)


def kernel(x, adj, W1, b1, W2, b2, W3, b3):
    cfg = Cfg()
    x = np.asarray(x)
    assert x.shape == (cfg.n, cfg.n_feat), x.shape
    if "nc" not in _NC_CACHE:
        _NC_CACHE["nc"] = build_nc(cfg)
    nc = _NC_CACHE["nc"]

    in_maps = shard_inputs(cfg, x, adj, W1, b1, W2)
    trace = os.environ.get("GCN_TRACE", "0") == "1"
    res = run_bass_kernel_spmd(
        nc, in_maps, core_ids=list(range(cfg.n_cores)), trace=trace)
    global LAST_RESULT
    LAST_RESULT = res
    per_core = np.stack(
        [np.asarray(r["out"][:, 0], dtype=np.float32) for r in res.results])
    return finish_on_host(cfg, per_core, b2, W3, b3)
